# revision 25
# baseline (speedup 1.0000x reference)
"""Trainium2 Bass kernel for an AttnBlock:
    y = x + proj( attention( qkv( groupnorm(x) ) ) )
with x [2, 512, 64, 64], 32-group GroupNorm, single-head spatial attention
over 4096 tokens with head dim 512, 1x1-conv Q/K/V/proj.

Sharding (8 cores): batch (2) x query-slice (4 x 1024 tokens).  Each core
computes GroupNorm stats + K and V^T for its whole batch image (redundantly
within its 4-core group, which is cheaper than communicating), and
Q / S^T=K^T.Q / softmax / P^T.V / proj only for its own 1024-query slice.

Layouts: channels-on-partitions [c, s] for xn/q/k, spatial-on-partitions
[j, c] for V^T, and S^T [j, i] so that softmax exp + sum run without any
transposes; the softmax normalizer 1/sum_j is folded in after the proj
matmul (proj is linear in columns), and the V bias is folded into the proj
bias on the host (softmax rows sum to 1).  Logits are bounded (|logit| <=
|q||k|/sqrt(512) ~ 23) so exp() needs no max-subtraction.  Matmuls run in
bf16 with fp32 PSUM accumulation; GroupNorm stats and all softmax
normalization stay fp32.
"""
import os
import sys

for _p in ("/opt/trn_rl_repo", "/root/.axon_site/_ro/trn_rl_repo"):
    if os.path.isdir(_p) and _p not in sys.path:
        sys.path.append(_p)

from contextlib import ExitStack

import numpy as np
import ml_dtypes

import concourse.bacc as bacc
import concourse.tile as tile
import concourse.mybir as mybir
from concourse.bass_utils import run_bass_kernel_spmd

F32 = mybir.dt.float32
BF16 = mybir.dt.bfloat16
FP8 = mybir.dt.float8e4
AF = mybir.ActivationFunctionType
OP = mybir.AluOpType
DR = mybir.MatmulPerfMode.DoubleRow

C = 512            # channels
S = 4096           # spatial tokens (64*64)
ISL = 1024         # query slice per core
NB = C // 128      # 4 channel blocks
NJC = S // 512     # 8 spatial 512-chunks
NCH = ISL // 512   # 2 query 512-chunks
NG = 32            # groupnorm groups
GPB = 128 // 16    # 8 groups per channel block
EPS = 1e-6
SCALE = float(C) ** -0.5
NCORES = 8


def declare_io(nc, fp8=False):
    T = {}
    T["x_bf"] = nc.dram_tensor("x_bf", [C, S], BF16, kind="ExternalInput")
    T["x_sl"] = nc.dram_tensor("x_sl", [C, ISL], F32, kind="ExternalInput")
    if fp8:
        # q/k/v weights in channel-paired DoubleRow layout [t2, p, i, c_out],
        # contraction channel = t2*256 + i*128 + p
        for w in ("wq8", "wk8", "wv8"):
            T[w] = nc.dram_tensor(w, [2, 128, 2, C], FP8, kind="ExternalInput")
        T["wpT"] = nc.dram_tensor("wpT", [C, C], BF16, kind="ExternalInput")
    else:
        for w in ("wqT", "wkT", "wvT", "wpT"):
            T[w] = nc.dram_tensor(w, [C, C], BF16, kind="ExternalInput")
    for v in ("gamma4", "beta4", "bq4", "bp24"):
        T[v] = nc.dram_tensor(v, [128, NB], F32, kind="ExternalInput")
    T["selr"] = nc.dram_tensor("selr", [128, GPB], F32, kind="ExternalInput")
    T["sele"] = nc.dram_tensor("sele", [GPB, 128], F32, kind="ExternalInput")
    return T


def emit_attn_block(nc, tc, T, out_d, rep="", gather_kv=False, fp8=False):
    """Emit one full per-core AttnBlock program into the TileContext.

    gather_kv: each core computes K/V^T only for its spatial quarter (the
    host rolls x so the core's quarter sits at columns 0:1024) and the four
    cores of a batch AllGather the full K/V^T; otherwise every core computes
    the full K/V^T redundantly.

    fp8: q/k/v convs and S^T=K^T.Q run in fp8(e4m3) DoubleRow perf mode
    (2x PE throughput, contraction pairs of channel blocks); P.V and proj
    stay bf16 (P would underflow fp8).
    """
    assert not (gather_kv and fp8)
    with ExitStack() as ctx:
        pc = ctx.enter_context(tc.tile_pool(name=rep + "const", bufs=1))
        pbig = ctx.enter_context(tc.tile_pool(name=rep + "big", bufs=1))
        pxn = ctx.enter_context(tc.tile_pool(name=rep + "xnp", bufs=3))
        pw = ctx.enter_context(tc.tile_pool(name=rep + "work", bufs=1))
        pps = ctx.enter_context(tc.tile_pool(name=rep + "psum", bufs=8, space="PSUM"))
        pdram = ctx.enter_context(tc.tile_pool(name=rep + "dram", bufs=1, space="DRAM"))

        def ps(nm):
            return pps.tile([128, 512], F32, name=rep + nm, tag="ps")

        # ---- big persistent tiles (x first: GroupNorm stats block on it) ----
        # two half-tile DMAs per block, split across both DGE paths, so
        # bn_stats can start on the first half while the second streams in
        xb = []      # x (bf16) [128, 4096] per channel block
        for t in range(NB):
            xt = pbig.tile([128, S], BF16, name=f"{rep}xb{t}")
            eng = nc.sync if t % 2 == 0 else nc.gpsimd
            eng.dma_start(out=xt[:, 0:S // 2], in_=T["x_bf"][t * 128:(t + 1) * 128, 0:S // 2])
            eng.dma_start(out=xt[:, S // 2:S], in_=T["x_bf"][t * 128:(t + 1) * 128, S // 2:S])
            xb.append(xt)

        # ---- constants ----
        ones_col = pc.tile([128, 1], F32, name=rep + "ones_col")
        nc.vector.memset(ones_col, 1.0)
        # ones_row via exp(0): also preloads the ACT Exp table so the first
        # softmax exp doesn't eat a ~4us LoadActFuncSet on the critical path
        ones_row = pc.tile([1, 128], F32, name=rep + "ones_row")
        nc.vector.memset(ones_row, 0.0)
        nc.scalar.activation(out=ones_row, in_=ones_row, func=AF.Exp, scale=1.0)
        eps8 = pc.tile([GPB, 1], F32, name=rep + "eps8")
        nc.vector.memset(eps8, EPS)

        selr_t = pc.tile([128, GPB], F32, name=rep + "selr_t")
        nc.sync.dma_start(out=selr_t, in_=T["selr"][:, :])
        sele_t = pc.tile([GPB, 128], F32, name=rep + "sele_t")
        nc.sync.dma_start(out=sele_t, in_=T["sele"][:, :])

        vec = {}
        for v in ("gamma4", "beta4", "bq4", "bp24"):
            vec[v] = pc.tile([128, NB], F32, name=rep + v)
            nc.sync.dma_start(out=vec[v], in_=T[v][:, :])

        xsl = []     # x slice (f32) [128, 1024] per channel block
        for t in range(NB):
            st = pbig.tile([128, ISL], F32, name=f"{rep}xsl{t}")
            nc.sync.dma_start(out=st, in_=T["x_sl"][t * 128:(t + 1) * 128, :])
            xsl.append(st)

        wt = {}
        for w in (("wpT",) if fp8 else ("wqT", "wkT", "wvT", "wpT")):
            wt[w] = []
            for t in range(NB):
                wtile = pbig.tile([128, C], BF16, name=f"{rep}{w}{t}")
                nc.sync.dma_start(out=wtile, in_=T[w][t * 128:(t + 1) * 128, :])
                wt[w].append(wtile)
        if fp8:
            for w in ("wq8", "wk8", "wv8"):
                wt[w] = []
                for t2 in range(2):
                    wtile = pbig.tile([128, 2, C], FP8, name=f"{rep}{w}{t2}")
                    nc.sync.dma_start(out=wtile, in_=T[w][t2, :, :, :])
                    wt[w].append(wtile)

        # ---- GroupNorm statistics ----
        # per-channel mean/var via bn_stats over the 4096 free dim
        stats_all = pw.tile([128, 2 * NB], F32, name=rep + "stats_all")
        for t in range(NB):
            bst = pw.tile([128, NJC, 6], F32, name=f"{rep}bnst{t}", tag="bnst", bufs=2)
            for sg in range(NJC):
                nc.vector.bn_stats(out=bst[:, sg, :], in_=xb[t][:, sg * 512:(sg + 1) * 512])
            nc.vector.bn_aggr(out=stats_all[:, 2 * t:2 * t + 2], in_=bst)
            # convert variance to E[x^2] = var + mean^2
            msq = pw.tile([128, 1], F32, name=f"{rep}msq{t}", tag="msq", bufs=2)
            nc.vector.tensor_mul(out=msq, in0=stats_all[:, 2 * t:2 * t + 1],
                                 in1=stats_all[:, 2 * t:2 * t + 1])
            nc.vector.tensor_add(out=stats_all[:, 2 * t + 1:2 * t + 2],
                                 in0=stats_all[:, 2 * t + 1:2 * t + 2], in1=msq)

        # reduce 16 channels -> group (selr holds 1/16 mask): [8, 2*NB]
        g_ps = pps.tile([GPB, 2 * NB], F32, name=rep + "g_ps", tag="ps")
        nc.tensor.matmul(g_ps, selr_t, stats_all, start=True, stop=True)

        # per-group mean / E[x^2] -> inv_std;  pack[:, 0:NB]=mean, [:, NB:]=inv
        pack = pw.tile([GPB, 2 * NB], F32, name=rep + "pack")
        gvar = pw.tile([GPB, NB], F32, name=rep + "gvar")
        nc.vector.tensor_copy(out=pack[:, 0:NB], in_=g_ps[:, 0:2 * NB:2])
        nc.vector.tensor_mul(out=gvar, in0=pack[:, 0:NB], in1=pack[:, 0:NB])
        nc.vector.tensor_sub(out=gvar, in0=g_ps[:, 1:2 * NB:2], in1=gvar)
        nc.scalar.activation(out=gvar, in_=gvar, func=AF.Sqrt, bias=eps8, scale=1.0)
        nc.vector.reciprocal(out=pack[:, NB:2 * NB], in_=gvar)

        # expand groups -> channels: [128, 2*NB]
        exp_ps = pps.tile([128, 2 * NB], F32, name=rep + "exp_ps", tag="ps")
        nc.tensor.matmul(exp_ps, sele_t, pack, start=True, stop=True)

        # per-channel affine xn = x*A + B  (gamma/beta folded in)
        A4 = pw.tile([128, NB], F32, name=rep + "A4")
        B4 = pw.tile([128, NB], F32, name=rep + "B4")
        nc.vector.tensor_mul(out=A4, in0=vec["gamma4"], in1=exp_ps[:, NB:2 * NB])
        nc.vector.tensor_mul(out=B4, in0=exp_ps[:, 0:NB], in1=A4)
        nc.vector.tensor_sub(out=B4, in0=vec["beta4"], in1=B4)

        # ---- normalized x chunks; in gather mode the core's query slice
        # equals its K/V quarter (rolled to columns 0:1024), so q reuses them.
        mm_dt = FP8 if fp8 else BF16
        if not gather_kv:
            # xnsl[t2][ch]: fp8 paired [128, 2, 512]; bf16 flat [t][ch] [128, 512]
            if fp8:
                xnsl = [[pbig.tile([128, 2, 512], FP8, name=f"{rep}xnsl{t2}_{ch}")
                         for ch in range(NCH)] for t2 in range(2)]
                for t2 in range(2):
                    for i in range(2):
                        t = 2 * t2 + i
                        for ch in range(NCH):
                            nc.vector.tensor_scalar(
                                out=xnsl[t2][ch][:, i, :],
                                in0=xsl[t][:, ch * 512:(ch + 1) * 512],
                                scalar1=A4[:, t:t + 1], scalar2=B4[:, t:t + 1],
                                op0=OP.mult, op1=OP.add)
            else:
                xnsl = []
                for t in range(NB):
                    row = []
                    for ch in range(NCH):
                        xt = pbig.tile([128, 512], BF16, name=f"{rep}xnsl{t}_{ch}")
                        nc.vector.tensor_scalar(
                            out=xt, in0=xsl[t][:, ch * 512:(ch + 1) * 512],
                            scalar1=A4[:, t:t + 1], scalar2=B4[:, t:t + 1],
                            op0=OP.mult, op1=OP.add)
                        row.append(xt)
                    xnsl.append(row)

        # ---- xn chunks, k, vT ----
        k = [[None] * NJC for _ in range(NB)]   # [t_out][jc] bf16 [128, 512]
        vt = [None] * (S // 128)                # [jb] bf16 [128, 512] = V^T[j, :]
        njc_loc = 2 if gather_kv else NJC       # quarter (1024 cols) vs full

        if gather_kv:
            snd_k = pdram.tile([C, ISL], BF16, name=rep + "snd_k")
            rcv_k = pdram.tile([4 * C, ISL], BF16, name=rep + "rcv_k")
            snd_v = pdram.tile([ISL, C], BF16, name=rep + "snd_v")
            rcv_v = pdram.tile([4 * ISL, C], BF16, name=rep + "rcv_v")

        xn_all = []
        for jc in range(njc_loc):
            if fp8:
                xn = [pxn.tile([128, 2, 512], FP8, name=f"{rep}xn{t2}_{jc}",
                               tag=f"xn{t2}") for t2 in range(2)]
                for t2 in range(2):
                    for i in range(2):
                        t = 2 * t2 + i
                        nc.vector.tensor_scalar(
                            out=xn[t2][:, i, :], in0=xb[t][:, jc * 512:(jc + 1) * 512],
                            scalar1=A4[:, t:t + 1], scalar2=B4[:, t:t + 1],
                            op0=OP.mult, op1=OP.add)
            else:
                xn = []
                for t in range(NB):
                    xt = pxn.tile([128, 512], BF16, name=f"{rep}xn{t}_{jc}", tag=f"xn{t}")
                    nc.vector.tensor_scalar(
                        out=xt, in0=xb[t][:, jc * 512:(jc + 1) * 512],
                        scalar1=A4[:, t:t + 1], scalar2=B4[:, t:t + 1],
                        op0=OP.mult, op1=OP.add)
                    xn.append(xt)
            xn_all.append(xn)

        # k2[t2][jc] (fp8): [128(c_out pair p), 2(i), 512(j)]
        k2 = [[None] * NJC for _ in range(2)]
        for jc in range(njc_loc):
            xn = xn_all[jc]
            if fp8:
                for t2 in range(2):
                    k2[t2][jc] = pbig.tile([128, 2, 512], FP8, name=f"{rep}k2_{t2}_{jc}")
            for t_out in range(NB):
                k_ps = ps(f"k_ps{t_out}_{jc}")
                if fp8:
                    for t2 in range(2):
                        nc.tensor.matmul(
                            k_ps, wt["wk8"][t2][:, :, t_out * 128:(t_out + 1) * 128],
                            xn[t2], start=(t2 == 0), stop=(t2 == 1), perf_mode=DR)
                    nc.scalar.copy(out=k2[t_out // 2][jc][:, t_out % 2, :], in_=k_ps)
                    continue
                for t in range(NB):
                    nc.tensor.matmul(
                        k_ps, wt["wkT"][t][:, t_out * 128:(t_out + 1) * 128],
                        xn[t], start=(t == 0), stop=(t == NB - 1))
                kt = pbig.tile([128, 512], BF16,
                               name=f"{rep}k{t_out}_{jc}",
                               tag=f"kloc" if gather_kv else "",
                               bufs=4 if gather_kv else 1)
                nc.scalar.copy(out=kt, in_=k_ps)
                if gather_kv:
                    nc.sync.dma_start(
                        out=snd_k[t_out * 128:(t_out + 1) * 128, jc * 512:(jc + 1) * 512],
                        in_=kt)
                else:
                    k[t_out][jc] = kt
            for jj in range(4):
                jb = jc * 4 + jj
                vt_ps = ps(f"vt_ps{jb}")
                if fp8:
                    for t2 in range(2):
                        nc.tensor.matmul(
                            vt_ps, xn[t2][:, :, jj * 128:(jj + 1) * 128],
                            wt["wv8"][t2], start=(t2 == 0), stop=(t2 == 1), perf_mode=DR)
                else:
                    for t in range(NB):
                        nc.tensor.matmul(
                            vt_ps, xn[t][:, jj * 128:(jj + 1) * 128],
                            wt["wvT"][t], start=(t == 0), stop=(t == NB - 1))
                vtt = pbig.tile([128, 512], BF16, name=f"{rep}vt{jb}",
                                tag=f"vtloc" if gather_kv else "",
                                bufs=4 if gather_kv else 1)
                nc.vector.tensor_copy(out=vtt, in_=vt_ps)
                if gather_kv:
                    nc.sync.dma_start(
                        out=snd_v[jb * 128:(jb + 1) * 128, :], in_=vtt)
                else:
                    vt[jb] = vtt

        if gather_kv:
            groups = [[0, 1, 2, 3], [4, 5, 6, 7]]
            nc.gpsimd.collective_compute(
                "AllGather", OP.bypass, replica_groups=groups,
                ins=[snd_k.opt()], outs=[rcv_k.opt()])
            nc.gpsimd.collective_compute(
                "AllGather", OP.bypass, replica_groups=groups,
                ins=[snd_v.opt()], outs=[rcv_v.opt()])

        # ---- q (runs on PE while the gather is in flight) ----
        q = [[None] * NCH for _ in range(NB)]  # [t_out][ch] bf16 [128, 512]
        q2 = [[None] * NCH for _ in range(2)]  # fp8 paired [128, 2, 512]
        if fp8:
            for t2 in range(2):
                for ch in range(NCH):
                    q2[t2][ch] = pbig.tile([128, 2, 512], FP8, name=f"{rep}q2_{t2}_{ch}")
        for t_out in range(NB):
            for ch in range(NCH):
                q_ps = ps(f"q_ps{t_out}_{ch}")
                if fp8:
                    for t2 in range(2):
                        nc.tensor.matmul(
                            q_ps, wt["wq8"][t2][:, :, t_out * 128:(t_out + 1) * 128],
                            xnsl[t2][ch], start=(t2 == 0), stop=(t2 == 1), perf_mode=DR)
                    nc.scalar.activation(
                        out=q2[t_out // 2][ch][:, t_out % 2, :], in_=q_ps,
                        func=AF.Identity, bias=vec["bq4"][:, t_out:t_out + 1], scale=1.0)
                    continue
                q_rhs = xn_all[ch] if gather_kv else [xnsl[t][ch] for t in range(NB)]
                for t in range(NB):
                    nc.tensor.matmul(
                        q_ps, wt["wqT"][t][:, t_out * 128:(t_out + 1) * 128],
                        q_rhs[t], start=(t == 0), stop=(t == NB - 1))
                qt = pbig.tile([128, 512], BF16, name=f"{rep}q{t_out}_{ch}")
                nc.scalar.activation(out=qt, in_=q_ps, func=AF.Identity,
                                     bias=vec["bq4"][:, t_out:t_out + 1], scale=1.0)
                q[t_out][ch] = qt

        if gather_kv:
            # unpack: quarter qq contributed true columns [qq*1024, qq*1024+1024)
            for qq in range(4):
                for jcl in range(2):
                    jc = qq * 2 + jcl
                    for t in range(NB):
                        kt = pbig.tile([128, 512], BF16, name=f"{rep}kg{t}_{jc}")
                        nc.sync.dma_start(
                            out=kt,
                            in_=rcv_k[qq * C + t * 128: qq * C + (t + 1) * 128,
                                      jcl * 512:(jcl + 1) * 512])
                        k[t][jc] = kt
                for jbl in range(8):
                    jb = qq * 8 + jbl
                    vtt = pbig.tile([128, 512], BF16, name=f"{rep}vtg{jb}")
                    nc.sync.dma_start(
                        out=vtt,
                        in_=rcv_v[qq * ISL + jbl * 128: qq * ISL + (jbl + 1) * 128, :])
                    vt[jb] = vtt

        # ---- attention + proj, one 512-query chunk at a time ----
        for ch in range(NCH):
            pv_ps = [ps(f"pv{t}_{ch}") for t in range(NB)]
            sums = pw.tile([128, 512], F32, name=f"{rep}sums{ch}", tag="sums", bufs=2)
            for jb in range(S // 128):
                st_ps = ps(f"st{jb}_{ch}")
                if fp8:
                    for t2 in range(2):
                        nc.tensor.matmul(
                            st_ps,
                            k2[t2][jb // 4][:, :, (jb % 4) * 128:(jb % 4 + 1) * 128],
                            q2[t2][ch], start=(t2 == 0), stop=(t2 == 1), perf_mode=DR)
                else:
                    for t in range(NB):
                        nc.tensor.matmul(
                            st_ps, k[t][jb // 4][:, (jb % 4) * 128:(jb % 4 + 1) * 128],
                            q[t][ch], start=(t == 0), stop=(t == NB - 1))
                pt = pw.tile([128, 512], BF16, name=f"{rep}pt{jb}_{ch}", tag="pt", bufs=4)
                nc.scalar.activation(out=pt, in_=st_ps, func=AF.Exp, scale=SCALE)
                # softmax-denominator partials accumulate on the otherwise-idle
                # gpsimd engine (SBUF-only traffic)
                if jb == 0:
                    nc.gpsimd.tensor_copy(out=sums, in_=pt)
                else:
                    nc.gpsimd.tensor_add(out=sums, in0=sums, in1=pt)
                for t in range(NB):
                    nc.tensor.matmul(pv_ps[t], vt[jb][:, t * 128:(t + 1) * 128],
                                     pt, start=(jb == 0), stop=(jb == S // 128 - 1))

            # softmax denominator -> reciprocal, broadcast to 128 partitions
            s_ps = pps.tile([1, 512], F32, name=f"{rep}s_ps{ch}", tag="ps")
            nc.tensor.matmul(s_ps, ones_col, sums, start=True, stop=True)
            recip = pw.tile([1, 512], F32, name=f"{rep}recip{ch}", tag="recip", bufs=2)
            nc.vector.reciprocal(out=recip, in_=s_ps)
            bc_ps = pps.tile([128, 512], F32, name=f"{rep}bc_ps{ch}", tag="ps")
            nc.tensor.matmul(bc_ps, ones_row, recip, start=True, stop=True)
            bc_sb = pw.tile([128, 512], F32, name=f"{rep}bc{ch}", tag="bcs", bufs=2)
            nc.vector.tensor_copy(out=bc_sb, in_=bc_ps)

            ao = []
            for t in range(NB):
                aot = pw.tile([128, 512], BF16, name=f"{rep}ao{t}_{ch}", tag="ao", bufs=4)
                nc.vector.tensor_copy(out=aot, in_=pv_ps[t])
                ao.append(aot)

            for t_out in range(NB):
                pj_ps = ps(f"pj{t_out}_{ch}")
                for t in range(NB):
                    nc.tensor.matmul(
                        pj_ps, wt["wpT"][t][:, t_out * 128:(t_out + 1) * 128],
                        ao[t], start=(t == 0), stop=(t == NB - 1))
                stg = pw.tile([128, 512], F32, name=f"{rep}stg{t_out}_{ch}", tag="stg", bufs=3)
                nc.vector.tensor_mul(out=stg, in0=pj_ps, in1=bc_sb)
                nc.vector.scalar_tensor_tensor(
                    out=stg, in0=stg, scalar=vec["bp24"][:, t_out:t_out + 1],
                    in1=xsl[t_out][:, ch * 512:(ch + 1) * 512],
                    op0=OP.add, op1=OP.add)
                nc.sync.dma_start(
                    out=out_d[t_out * 128:(t_out + 1) * 128, ch * 512:(ch + 1) * 512],
                    in_=stg)


GATHER_KV = False  # measured: in-group AllGather costs more than redundant K/V here
USE_FP8 = True     # fp8(e4m3) DoubleRow for q/k/v convs + S^T


def build_program(nreps=1, gather_kv=GATHER_KV, fp8=USE_FP8):
    nc = bacc.Bacc("TRN2", target_bir_lowering=False, debug=False,
                   num_devices=NCORES)
    T = declare_io(nc, fp8=fp8)
    out_d = nc.dram_tensor("out", [C, ISL], F32, kind="ExternalOutput")
    with tile.TileContext(nc) as tc:
        for r in range(nreps):
            emit_attn_block(nc, tc, T, out_d, rep=f"r{r}_" if nreps > 1 else "",
                            gather_kv=gather_kv, fp8=fp8)
    nc.compile()
    return nc


_NC_CACHE = {}


def get_program(nreps=1, gather_kv=GATHER_KV, fp8=USE_FP8):
    key = (nreps, gather_kv, fp8)
    if key not in _NC_CACHE:
        _NC_CACHE[key] = build_program(nreps, gather_kv, fp8)
    return _NC_CACHE[key]


def make_in_maps(x, gn_w, gn_b, wq, bq, wk, bk, wv, bv, wp, bp):
    bf16 = ml_dtypes.bfloat16
    B = x.shape[0]
    xr = np.ascontiguousarray(np.asarray(x, np.float32).reshape(B, C, S))
    xbf = xr.astype(bf16)

    def v4(v):
        return np.ascontiguousarray(np.asarray(v, np.float32).reshape(NB, 128).T)

    # fold the V bias through the proj (softmax rows sum to 1):
    #   proj(attn_out + bv) = proj(attn_out) + wp @ bv
    bp2 = (np.asarray(bp, np.float64)
           + np.asarray(wp, np.float64) @ np.asarray(bv, np.float64)).astype(np.float32)

    p = np.arange(128)
    selr = np.zeros((128, GPB), np.float32)
    selr[p, p // 16] = 1.0 / 16.0
    sele = np.zeros((GPB, 128), np.float32)
    sele[p // 16, p] = 1.0

    shared = {
        "wpT": np.ascontiguousarray(np.asarray(wp, np.float32).T).astype(bf16),
        "gamma4": v4(gn_w), "beta4": v4(gn_b), "bq4": v4(bq), "bp24": v4(bp2),
        "selr": selr, "sele": sele,
    }
    if USE_FP8:
        f8 = ml_dtypes.float8_e4m3fn

        def pair8(w):
            # w.T [c_in, c_out] -> [t2, p, i, c_out] with c_in = t2*256+i*128+p
            wT = np.asarray(w, np.float32).T.reshape(2, 2, 128, C)
            return np.ascontiguousarray(wT.transpose(0, 2, 1, 3)).astype(f8)

        shared["wq8"] = pair8(wq)
        shared["wk8"] = pair8(wk)
        shared["wv8"] = pair8(wv)
    else:
        shared["wqT"] = np.ascontiguousarray(np.asarray(wq, np.float32).T).astype(bf16)
        shared["wkT"] = np.ascontiguousarray(np.asarray(wk, np.float32).T).astype(bf16)
        shared["wvT"] = np.ascontiguousarray(np.asarray(wv, np.float32).T).astype(bf16)
    in_maps = []
    for core in range(NCORES):
        b = core // 4
        i0 = (core % 4) * ISL
        m = dict(shared)
        if GATHER_KV:
            # roll so this core's spatial quarter sits at columns 0:1024 --
            # the (identical SPMD) program computes K/V^T on columns 0:1024
            # and the in-group AllGather reassembles them in true order.
            m["x_bf"] = np.roll(xbf[b], -i0, axis=1)
        else:
            m["x_bf"] = xbf[b]
        m["x_sl"] = np.ascontiguousarray(xr[b][:, i0:i0 + ISL])
        in_maps.append(m)
    return in_maps


def kernel(x, gn_w, gn_b, wq, bq, wk, bk, wv, bv, wp, bp):
    x = np.asarray(x)
    B = x.shape[0]
    nc = get_program(1)
    in_maps = make_in_maps(x, gn_w, gn_b, wq, bq, wk, bk, wv, bv, wp, bp)
    res = run_bass_kernel_spmd(nc, in_maps, core_ids=list(range(NCORES)))
    out = np.empty((B, C, S), np.float32)
    for core in range(NCORES):
        b = core // 4
        i0 = (core % 4) * ISL
        out[b][:, i0:i0 + ISL] = res.results[core]["out"]
    return out.reshape(x.shape).astype(np.float32)


# revision 28
# speedup vs baseline: 1.8851x; 1.8851x over previous
"""Trainium2 Bass kernel for an AttnBlock:
    y = x + proj( attention( qkv( groupnorm(x) ) ) )
with x [2, 512, 64, 64], 32-group GroupNorm, single-head spatial attention
over 4096 tokens with head dim 512, 1x1-conv Q/K/V/proj.

Sharding (8 cores): batch (2) x query-slice (4 x 1024 tokens).  Each core
computes GroupNorm stats + K and V^T for its whole batch image (redundantly
within its 4-core group, which is cheaper than communicating), and
Q / S^T=K^T.Q / softmax / P^T.V / proj only for its own 1024-query slice.

Layouts: channels-on-partitions [c, s] for xn/q/k, spatial-on-partitions
[j, c] for V^T, and S^T [j, i] so that softmax exp + sum run without any
transposes; the softmax normalizer 1/sum_j is folded in after the proj
matmul (proj is linear in columns), and the V bias is folded into the proj
bias on the host (softmax rows sum to 1).  Logits are bounded (|logit| <=
|q||k|/sqrt(512) ~ 23) so exp() needs no max-subtraction.  Matmuls run in
bf16 with fp32 PSUM accumulation; GroupNorm stats and all softmax
normalization stay fp32.
"""
import os
import sys

for _p in ("/opt/trn_rl_repo", "/root/.axon_site/_ro/trn_rl_repo"):
    if os.path.isdir(_p) and _p not in sys.path:
        sys.path.append(_p)

from contextlib import ExitStack

import numpy as np
import ml_dtypes

import concourse.bacc as bacc
import concourse.tile as tile
import concourse.mybir as mybir
from concourse.bass_utils import run_bass_kernel_spmd

F32 = mybir.dt.float32
BF16 = mybir.dt.bfloat16
FP8 = mybir.dt.float8e4
AF = mybir.ActivationFunctionType
OP = mybir.AluOpType
DR = mybir.MatmulPerfMode.DoubleRow

C = 512            # channels
S = 4096           # spatial tokens (64*64)
ISL = 1024         # query slice per core
NB = C // 128      # 4 channel blocks
NJC = S // 512     # 8 spatial 512-chunks
NCH = ISL // 512   # 2 query 512-chunks
NG = 32            # groupnorm groups
GPB = 128 // 16    # 8 groups per channel block
EPS = 1e-6
SCALE = float(C) ** -0.5
NCORES = 8


def declare_io(nc, fp8=False):
    T = {}
    T["x_bf"] = nc.dram_tensor("x_bf", [C, S], BF16, kind="ExternalInput")
    T["x_sl"] = nc.dram_tensor("x_sl", [C, ISL], F32, kind="ExternalInput")
    if fp8:
        # q/k/v weights in channel-paired DoubleRow layout [t2, p, i, c_out],
        # contraction channel = t2*256 + i*128 + p
        for w in ("wq8", "wk8", "wv8"):
            T[w] = nc.dram_tensor(w, [2, 128, 2, C], FP8, kind="ExternalInput")
        T["wpT"] = nc.dram_tensor("wpT", [C, C], BF16, kind="ExternalInput")
    else:
        for w in ("wqT", "wkT", "wvT", "wpT"):
            T[w] = nc.dram_tensor(w, [C, C], BF16, kind="ExternalInput")
    for v in ("gamma4", "beta4", "bq4", "bp24"):
        T[v] = nc.dram_tensor(v, [128, NB], F32, kind="ExternalInput")
    T["selr"] = nc.dram_tensor("selr", [128, GPB], F32, kind="ExternalInput")
    T["sele"] = nc.dram_tensor("sele", [GPB, 128], F32, kind="ExternalInput")
    return T


def emit_attn_block(nc, tc, T, out_d, rep="", gather_kv=False, fp8=False):
    """Emit one full per-core AttnBlock program into the TileContext.

    gather_kv: each core computes K/V^T only for its spatial quarter (the
    host rolls x so the core's quarter sits at columns 0:1024) and the four
    cores of a batch AllGather the full K/V^T; otherwise every core computes
    the full K/V^T redundantly.

    fp8: q/k/v convs and S^T=K^T.Q run in fp8(e4m3) DoubleRow perf mode
    (2x PE throughput, contraction pairs of channel blocks); P.V and proj
    stay bf16 (P would underflow fp8).
    """
    assert not (gather_kv and fp8)
    with ExitStack() as ctx:
        pc = ctx.enter_context(tc.tile_pool(name=rep + "const", bufs=1))
        pbig = ctx.enter_context(tc.tile_pool(name=rep + "big", bufs=1))
        pxn = ctx.enter_context(tc.tile_pool(name=rep + "xnp", bufs=3))
        pw = ctx.enter_context(tc.tile_pool(name=rep + "work", bufs=1))
        pps = ctx.enter_context(tc.tile_pool(name=rep + "psum", bufs=8, space="PSUM"))
        pdram = ctx.enter_context(tc.tile_pool(name=rep + "dram", bufs=1, space="DRAM"))

        def ps(nm):
            return pps.tile([128, 512], F32, name=rep + nm, tag="ps")

        # ---- big persistent tiles (x first: GroupNorm stats block on it) ----
        # two half-tile DMAs per block, split across both DGE paths, so
        # bn_stats can start on the first half while the second streams in
        xb = []      # x (bf16) [128, 4096] per channel block
        for t in range(NB):
            xt = pbig.tile([128, S], BF16, name=f"{rep}xb{t}")
            eng = nc.sync if t % 2 == 0 else nc.gpsimd
            eng.dma_start(out=xt[:, 0:S // 2], in_=T["x_bf"][t * 128:(t + 1) * 128, 0:S // 2])
            eng.dma_start(out=xt[:, S // 2:S], in_=T["x_bf"][t * 128:(t + 1) * 128, S // 2:S])
            xb.append(xt)

        # ---- constants ----
        ones_col = pc.tile([128, 1], F32, name=rep + "ones_col")
        nc.vector.memset(ones_col, 1.0)
        # ones_row via exp(0): also preloads the ACT Exp table so the first
        # softmax exp doesn't eat a ~4us LoadActFuncSet on the critical path
        ones_row = pc.tile([1, 128], F32, name=rep + "ones_row")
        nc.vector.memset(ones_row, 0.0)
        nc.scalar.activation(out=ones_row, in_=ones_row, func=AF.Exp, scale=1.0)
        eps8 = pc.tile([GPB, 1], F32, name=rep + "eps8")
        nc.vector.memset(eps8, EPS)

        selr_t = pc.tile([128, GPB], F32, name=rep + "selr_t")
        nc.sync.dma_start(out=selr_t, in_=T["selr"][:, :])
        sele_t = pc.tile([GPB, 128], F32, name=rep + "sele_t")
        nc.sync.dma_start(out=sele_t, in_=T["sele"][:, :])

        vec = {}
        for v in ("gamma4", "beta4", "bq4", "bp24"):
            vec[v] = pc.tile([128, NB], F32, name=rep + v)
            nc.sync.dma_start(out=vec[v], in_=T[v][:, :])

        xsl = []     # x slice (f32) [128, 1024] per channel block
        for t in range(NB):
            st = pbig.tile([128, ISL], F32, name=f"{rep}xsl{t}")
            nc.sync.dma_start(out=st, in_=T["x_sl"][t * 128:(t + 1) * 128, :])
            xsl.append(st)

        wt = {}
        for w in (("wpT",) if fp8 else ("wqT", "wkT", "wvT", "wpT")):
            wt[w] = []
            for t in range(NB):
                wtile = pbig.tile([128, C], BF16, name=f"{rep}{w}{t}")
                nc.sync.dma_start(out=wtile, in_=T[w][t * 128:(t + 1) * 128, :])
                wt[w].append(wtile)
        if fp8:
            for w in ("wq8", "wk8", "wv8"):
                wt[w] = []
                for t2 in range(2):
                    wtile = pbig.tile([128, 2, C], FP8, name=f"{rep}{w}{t2}")
                    nc.sync.dma_start(out=wtile, in_=T[w][t2, :, :, :])
                    wt[w].append(wtile)

        # ---- GroupNorm statistics ----
        # per-channel mean/var via bn_stats over the 4096 free dim
        stats_all = pw.tile([128, 2 * NB], F32, name=rep + "stats_all")
        for t in range(NB):
            bst = pw.tile([128, NJC, 6], F32, name=f"{rep}bnst{t}", tag="bnst", bufs=2)
            for sg in range(NJC):
                nc.vector.bn_stats(out=bst[:, sg, :], in_=xb[t][:, sg * 512:(sg + 1) * 512])
            nc.vector.bn_aggr(out=stats_all[:, 2 * t:2 * t + 2], in_=bst)
            # convert variance to E[x^2] = var + mean^2
            msq = pw.tile([128, 1], F32, name=f"{rep}msq{t}", tag="msq", bufs=2)
            nc.vector.tensor_mul(out=msq, in0=stats_all[:, 2 * t:2 * t + 1],
                                 in1=stats_all[:, 2 * t:2 * t + 1])
            nc.vector.tensor_add(out=stats_all[:, 2 * t + 1:2 * t + 2],
                                 in0=stats_all[:, 2 * t + 1:2 * t + 2], in1=msq)

        # reduce 16 channels -> group (selr holds 1/16 mask): [8, 2*NB]
        g_ps = pps.tile([GPB, 2 * NB], F32, name=rep + "g_ps", tag="ps")
        nc.tensor.matmul(g_ps, selr_t, stats_all, start=True, stop=True)

        # per-group mean / E[x^2] -> inv_std;  pack[:, 0:NB]=mean, [:, NB:]=inv
        pack = pw.tile([GPB, 2 * NB], F32, name=rep + "pack")
        gvar = pw.tile([GPB, NB], F32, name=rep + "gvar")
        nc.vector.tensor_copy(out=pack[:, 0:NB], in_=g_ps[:, 0:2 * NB:2])
        nc.vector.tensor_mul(out=gvar, in0=pack[:, 0:NB], in1=pack[:, 0:NB])
        nc.vector.tensor_sub(out=gvar, in0=g_ps[:, 1:2 * NB:2], in1=gvar)
        nc.scalar.activation(out=gvar, in_=gvar, func=AF.Sqrt, bias=eps8, scale=1.0)
        nc.vector.reciprocal(out=pack[:, NB:2 * NB], in_=gvar)

        # expand groups -> channels: [128, 2*NB]
        exp_ps = pps.tile([128, 2 * NB], F32, name=rep + "exp_ps", tag="ps")
        nc.tensor.matmul(exp_ps, sele_t, pack, start=True, stop=True)

        # per-channel affine xn = x*A + B  (gamma/beta folded in)
        A4 = pw.tile([128, NB], F32, name=rep + "A4")
        B4 = pw.tile([128, NB], F32, name=rep + "B4")
        nc.vector.tensor_mul(out=A4, in0=vec["gamma4"], in1=exp_ps[:, NB:2 * NB])
        nc.vector.tensor_mul(out=B4, in0=exp_ps[:, 0:NB], in1=A4)
        nc.vector.tensor_sub(out=B4, in0=vec["beta4"], in1=B4)

        # ---- normalized x chunks; in gather mode the core's query slice
        # equals its K/V quarter (rolled to columns 0:1024), so q reuses them.
        mm_dt = FP8 if fp8 else BF16
        if not gather_kv:
            # xnsl[t2][ch]: fp8 paired [128, 2, 512]; bf16 flat [t][ch] [128, 512]
            if fp8:
                xnsl = [[pbig.tile([128, 2, 512], FP8, name=f"{rep}xnsl{t2}_{ch}")
                         for ch in range(NCH)] for t2 in range(2)]
                for t2 in range(2):
                    for i in range(2):
                        t = 2 * t2 + i
                        for ch in range(NCH):
                            nc.vector.tensor_scalar(
                                out=xnsl[t2][ch][:, i, :],
                                in0=xsl[t][:, ch * 512:(ch + 1) * 512],
                                scalar1=A4[:, t:t + 1], scalar2=B4[:, t:t + 1],
                                op0=OP.mult, op1=OP.add)
            else:
                xnsl = []
                for t in range(NB):
                    row = []
                    for ch in range(NCH):
                        xt = pbig.tile([128, 512], BF16, name=f"{rep}xnsl{t}_{ch}")
                        nc.vector.tensor_scalar(
                            out=xt, in0=xsl[t][:, ch * 512:(ch + 1) * 512],
                            scalar1=A4[:, t:t + 1], scalar2=B4[:, t:t + 1],
                            op0=OP.mult, op1=OP.add)
                        row.append(xt)
                    xnsl.append(row)

        # ---- xn chunks, k, vT ----
        k = [[None] * NJC for _ in range(NB)]   # [t_out][jc] bf16 [128, 512]
        vt = [None] * (S // 128)                # [jb] bf16 [128, 512] = V^T[j, :]
        njc_loc = 2 if gather_kv else NJC       # quarter (1024 cols) vs full

        if gather_kv:
            snd_k = pdram.tile([C, ISL], BF16, name=rep + "snd_k")
            rcv_k = pdram.tile([4 * C, ISL], BF16, name=rep + "rcv_k")
            snd_v = pdram.tile([ISL, C], BF16, name=rep + "snd_v")
            rcv_v = pdram.tile([4 * ISL, C], BF16, name=rep + "rcv_v")

        xn_all = []
        for jc in range(njc_loc):
            if fp8:
                xn = [pxn.tile([128, 2, 512], FP8, name=f"{rep}xn{t2}_{jc}",
                               tag=f"xn{t2}") for t2 in range(2)]
                for t2 in range(2):
                    for i in range(2):
                        t = 2 * t2 + i
                        nc.vector.tensor_scalar(
                            out=xn[t2][:, i, :], in0=xb[t][:, jc * 512:(jc + 1) * 512],
                            scalar1=A4[:, t:t + 1], scalar2=B4[:, t:t + 1],
                            op0=OP.mult, op1=OP.add)
            else:
                xn = []
                for t in range(NB):
                    xt = pxn.tile([128, 512], BF16, name=f"{rep}xn{t}_{jc}", tag=f"xn{t}")
                    nc.vector.tensor_scalar(
                        out=xt, in0=xb[t][:, jc * 512:(jc + 1) * 512],
                        scalar1=A4[:, t:t + 1], scalar2=B4[:, t:t + 1],
                        op0=OP.mult, op1=OP.add)
                    xn.append(xt)
            xn_all.append(xn)

        # k2[t2][jc] (fp8): [128(c_out pair p), 2(i), 512(j)]
        k2 = [[None] * NJC for _ in range(2)]
        for jc in range(njc_loc):
            xn = xn_all[jc]
            if fp8:
                for t2 in range(2):
                    k2[t2][jc] = pbig.tile([128, 2, 512], FP8, name=f"{rep}k2_{t2}_{jc}")
            for t_out in range(NB):
                k_ps = ps(f"k_ps{t_out}_{jc}")
                if fp8:
                    for t2 in range(2):
                        nc.tensor.matmul(
                            k_ps, wt["wk8"][t2][:, :, t_out * 128:(t_out + 1) * 128],
                            xn[t2], start=(t2 == 0), stop=(t2 == 1), perf_mode=DR)
                    nc.scalar.copy(out=k2[t_out // 2][jc][:, t_out % 2, :], in_=k_ps)
                    continue
                for t in range(NB):
                    nc.tensor.matmul(
                        k_ps, wt["wkT"][t][:, t_out * 128:(t_out + 1) * 128],
                        xn[t], start=(t == 0), stop=(t == NB - 1))
                kt = pbig.tile([128, 512], BF16,
                               name=f"{rep}k{t_out}_{jc}",
                               tag=f"kloc" if gather_kv else "",
                               bufs=4 if gather_kv else 1)
                nc.scalar.copy(out=kt, in_=k_ps)
                if gather_kv:
                    nc.sync.dma_start(
                        out=snd_k[t_out * 128:(t_out + 1) * 128, jc * 512:(jc + 1) * 512],
                        in_=kt)
                else:
                    k[t_out][jc] = kt
            for jj in range(4):
                jb = jc * 4 + jj
                vt_ps = ps(f"vt_ps{jb}")
                if fp8:
                    for t2 in range(2):
                        nc.tensor.matmul(
                            vt_ps, xn[t2][:, :, jj * 128:(jj + 1) * 128],
                            wt["wv8"][t2], start=(t2 == 0), stop=(t2 == 1), perf_mode=DR)
                else:
                    for t in range(NB):
                        nc.tensor.matmul(
                            vt_ps, xn[t][:, jj * 128:(jj + 1) * 128],
                            wt["wvT"][t], start=(t == 0), stop=(t == NB - 1))
                vtt = pbig.tile([128, 512], BF16, name=f"{rep}vt{jb}",
                                tag=f"vtloc" if gather_kv else "",
                                bufs=4 if gather_kv else 1)
                nc.vector.tensor_copy(out=vtt, in_=vt_ps)
                if gather_kv:
                    nc.sync.dma_start(
                        out=snd_v[jb * 128:(jb + 1) * 128, :], in_=vtt)
                else:
                    vt[jb] = vtt

        if gather_kv:
            groups = [[0, 1, 2, 3], [4, 5, 6, 7]]
            nc.gpsimd.collective_compute(
                "AllGather", OP.bypass, replica_groups=groups,
                ins=[snd_k.opt()], outs=[rcv_k.opt()])
            nc.gpsimd.collective_compute(
                "AllGather", OP.bypass, replica_groups=groups,
                ins=[snd_v.opt()], outs=[rcv_v.opt()])

        # ---- q (runs on PE while the gather is in flight) ----
        q = [[None] * NCH for _ in range(NB)]  # [t_out][ch] bf16 [128, 512]
        q2 = [[None] * NCH for _ in range(2)]  # fp8 paired [128, 2, 512]
        if fp8:
            for t2 in range(2):
                for ch in range(NCH):
                    q2[t2][ch] = pbig.tile([128, 2, 512], FP8, name=f"{rep}q2_{t2}_{ch}")
        for t_out in range(NB):
            for ch in range(NCH):
                q_ps = ps(f"q_ps{t_out}_{ch}")
                if fp8:
                    for t2 in range(2):
                        nc.tensor.matmul(
                            q_ps, wt["wq8"][t2][:, :, t_out * 128:(t_out + 1) * 128],
                            xnsl[t2][ch], start=(t2 == 0), stop=(t2 == 1), perf_mode=DR)
                    nc.scalar.activation(
                        out=q2[t_out // 2][ch][:, t_out % 2, :], in_=q_ps,
                        func=AF.Identity, bias=vec["bq4"][:, t_out:t_out + 1], scale=1.0)
                    continue
                q_rhs = xn_all[ch] if gather_kv else [xnsl[t][ch] for t in range(NB)]
                for t in range(NB):
                    nc.tensor.matmul(
                        q_ps, wt["wqT"][t][:, t_out * 128:(t_out + 1) * 128],
                        q_rhs[t], start=(t == 0), stop=(t == NB - 1))
                qt = pbig.tile([128, 512], BF16, name=f"{rep}q{t_out}_{ch}")
                nc.scalar.activation(out=qt, in_=q_ps, func=AF.Identity,
                                     bias=vec["bq4"][:, t_out:t_out + 1], scale=1.0)
                q[t_out][ch] = qt

        if gather_kv:
            # unpack: quarter qq contributed true columns [qq*1024, qq*1024+1024)
            for qq in range(4):
                for jcl in range(2):
                    jc = qq * 2 + jcl
                    for t in range(NB):
                        kt = pbig.tile([128, 512], BF16, name=f"{rep}kg{t}_{jc}")
                        nc.sync.dma_start(
                            out=kt,
                            in_=rcv_k[qq * C + t * 128: qq * C + (t + 1) * 128,
                                      jcl * 512:(jcl + 1) * 512])
                        k[t][jc] = kt
                for jbl in range(8):
                    jb = qq * 8 + jbl
                    vtt = pbig.tile([128, 512], BF16, name=f"{rep}vtg{jb}")
                    nc.sync.dma_start(
                        out=vtt,
                        in_=rcv_v[qq * ISL + jbl * 128: qq * ISL + (jbl + 1) * 128, :])
                    vt[jb] = vtt

        # ---- attention + proj, one 512-query chunk at a time ----
        for ch in range(NCH):
            pv_ps = [ps(f"pv{t}_{ch}") for t in range(NB)]
            # two independent accumulation chains (even/odd jb) halve the
            # serial latency of the gpsimd adds; combined below
            sums = [pw.tile([128, 512], F32, name=f"{rep}sums{ch}_{par}",
                            tag=f"sums{par}", bufs=2) for par in range(2)]
            for jb in range(S // 128):
                st_ps = ps(f"st{jb}_{ch}")
                if fp8:
                    for t2 in range(2):
                        nc.tensor.matmul(
                            st_ps,
                            k2[t2][jb // 4][:, :, (jb % 4) * 128:(jb % 4 + 1) * 128],
                            q2[t2][ch], start=(t2 == 0), stop=(t2 == 1), perf_mode=DR)
                else:
                    for t in range(NB):
                        nc.tensor.matmul(
                            st_ps, k[t][jb // 4][:, (jb % 4) * 128:(jb % 4 + 1) * 128],
                            q[t][ch], start=(t == 0), stop=(t == NB - 1))
                pt = pw.tile([128, 512], BF16, name=f"{rep}pt{jb}_{ch}", tag="pt", bufs=4)
                nc.scalar.activation(out=pt, in_=st_ps, func=AF.Exp, scale=SCALE)
                # softmax-denominator partials accumulate on the otherwise-idle
                # gpsimd engine (SBUF-only traffic)
                sp = sums[jb % 2]
                if jb < 2:
                    nc.gpsimd.tensor_copy(out=sp, in_=pt)
                else:
                    nc.gpsimd.tensor_add(out=sp, in0=sp, in1=pt)
                for t in range(NB):
                    nc.tensor.matmul(pv_ps[t], vt[jb][:, t * 128:(t + 1) * 128],
                                     pt, start=(jb == 0), stop=(jb == S // 128 - 1))

            # softmax denominator -> reciprocal, broadcast to 128 partitions
            nc.vector.tensor_add(out=sums[0], in0=sums[0], in1=sums[1])
            s_ps = pps.tile([1, 512], F32, name=f"{rep}s_ps{ch}", tag="ps")
            nc.tensor.matmul(s_ps, ones_col, sums[0], start=True, stop=True)
            recip = pw.tile([1, 512], F32, name=f"{rep}recip{ch}", tag="recip", bufs=2)
            nc.vector.reciprocal(out=recip, in_=s_ps)
            bc_ps = pps.tile([128, 512], F32, name=f"{rep}bc_ps{ch}", tag="ps")
            nc.tensor.matmul(bc_ps, ones_row, recip, start=True, stop=True)
            bc_sb = pw.tile([128, 512], F32, name=f"{rep}bc{ch}", tag="bcs", bufs=2)
            nc.vector.tensor_copy(out=bc_sb, in_=bc_ps)

            ao = []
            for t in range(NB):
                aot = pw.tile([128, 512], BF16, name=f"{rep}ao{t}_{ch}", tag="ao", bufs=4)
                nc.vector.tensor_copy(out=aot, in_=pv_ps[t])
                ao.append(aot)

            for t_out in range(NB):
                pj_ps = ps(f"pj{t_out}_{ch}")
                for t in range(NB):
                    nc.tensor.matmul(
                        pj_ps, wt["wpT"][t][:, t_out * 128:(t_out + 1) * 128],
                        ao[t], start=(t == 0), stop=(t == NB - 1))
                stg = pw.tile([128, 512], F32, name=f"{rep}stg{t_out}_{ch}", tag="stg", bufs=3)
                nc.vector.tensor_mul(out=stg, in0=pj_ps, in1=bc_sb)
                nc.vector.scalar_tensor_tensor(
                    out=stg, in0=stg, scalar=vec["bp24"][:, t_out:t_out + 1],
                    in1=xsl[t_out][:, ch * 512:(ch + 1) * 512],
                    op0=OP.add, op1=OP.add)
                nc.sync.dma_start(
                    out=out_d[t_out * 128:(t_out + 1) * 128, ch * 512:(ch + 1) * 512],
                    in_=stg)


GATHER_KV = False  # measured: in-group AllGather costs more than redundant K/V here
USE_FP8 = True     # fp8(e4m3) DoubleRow for q/k/v convs + S^T


def build_program(nreps=1, gather_kv=GATHER_KV, fp8=USE_FP8):
    nc = bacc.Bacc("TRN2", target_bir_lowering=False, debug=False,
                   num_devices=NCORES)
    T = declare_io(nc, fp8=fp8)
    out_d = nc.dram_tensor("out", [C, ISL], F32, kind="ExternalOutput")
    with tile.TileContext(nc) as tc:
        for r in range(nreps):
            emit_attn_block(nc, tc, T, out_d, rep=f"r{r}_" if nreps > 1 else "",
                            gather_kv=gather_kv, fp8=fp8)
    nc.compile()
    return nc


_NC_CACHE = {}


def get_program(nreps=1, gather_kv=GATHER_KV, fp8=USE_FP8):
    key = (nreps, gather_kv, fp8)
    if key not in _NC_CACHE:
        _NC_CACHE[key] = build_program(nreps, gather_kv, fp8)
    return _NC_CACHE[key]


def make_in_maps(x, gn_w, gn_b, wq, bq, wk, bk, wv, bv, wp, bp):
    bf16 = ml_dtypes.bfloat16
    B = x.shape[0]
    xr = np.ascontiguousarray(np.asarray(x, np.float32).reshape(B, C, S))
    xbf = xr.astype(bf16)

    def v4(v):
        return np.ascontiguousarray(np.asarray(v, np.float32).reshape(NB, 128).T)

    # fold the V bias through the proj (softmax rows sum to 1):
    #   proj(attn_out + bv) = proj(attn_out) + wp @ bv
    bp2 = (np.asarray(bp, np.float64)
           + np.asarray(wp, np.float64) @ np.asarray(bv, np.float64)).astype(np.float32)

    p = np.arange(128)
    selr = np.zeros((128, GPB), np.float32)
    selr[p, p // 16] = 1.0 / 16.0
    sele = np.zeros((GPB, 128), np.float32)
    sele[p // 16, p] = 1.0

    shared = {
        "wpT": np.ascontiguousarray(np.asarray(wp, np.float32).T).astype(bf16),
        "gamma4": v4(gn_w), "beta4": v4(gn_b), "bq4": v4(bq), "bp24": v4(bp2),
        "selr": selr, "sele": sele,
    }
    if USE_FP8:
        f8 = ml_dtypes.float8_e4m3fn

        def pair8(w):
            # w.T [c_in, c_out] -> [t2, p, i, c_out] with c_in = t2*256+i*128+p
            wT = np.asarray(w, np.float32).T.reshape(2, 2, 128, C)
            return np.ascontiguousarray(wT.transpose(0, 2, 1, 3)).astype(f8)

        shared["wq8"] = pair8(wq)
        shared["wk8"] = pair8(wk)
        shared["wv8"] = pair8(wv)
    else:
        shared["wqT"] = np.ascontiguousarray(np.asarray(wq, np.float32).T).astype(bf16)
        shared["wkT"] = np.ascontiguousarray(np.asarray(wk, np.float32).T).astype(bf16)
        shared["wvT"] = np.ascontiguousarray(np.asarray(wv, np.float32).T).astype(bf16)
    in_maps = []
    for core in range(NCORES):
        b = core // 4
        i0 = (core % 4) * ISL
        m = dict(shared)
        if GATHER_KV:
            # roll so this core's spatial quarter sits at columns 0:1024 --
            # the (identical SPMD) program computes K/V^T on columns 0:1024
            # and the in-group AllGather reassembles them in true order.
            m["x_bf"] = np.roll(xbf[b], -i0, axis=1)
        else:
            m["x_bf"] = xbf[b]
        m["x_sl"] = np.ascontiguousarray(xr[b][:, i0:i0 + ISL])
        in_maps.append(m)
    return in_maps


def kernel(x, gn_w, gn_b, wq, bq, wk, bk, wv, bv, wp, bp):
    x = np.asarray(x)
    B = x.shape[0]
    nc = get_program(1)
    in_maps = make_in_maps(x, gn_w, gn_b, wq, bq, wk, bk, wv, bv, wp, bp)
    res = run_bass_kernel_spmd(nc, in_maps, core_ids=list(range(NCORES)))
    out = np.empty((B, C, S), np.float32)
    for core in range(NCORES):
        b = core // 4
        i0 = (core % 4) * ISL
        out[b][:, i0:i0 + ISL] = res.results[core]["out"]
    return out.reshape(x.shape).astype(np.float32)
